# revision 39
# baseline (speedup 1.0000x reference)
"""Trainium2 Bass kernel for nn_DilatedAttentionBlock_attention.

Per-core work (data-parallel over batch, 8 cores):
  x [C=256, L=2048] -> QKV MLPs -> 4-head attention with Lipschitz score
  rescale -> out-proj -> LayerNorm -> ELU + residual -> out [C, L].

Everything stays in channel-major ("transposed") [C, L] layout, which is the
native layout of x_in, so weights act as natural lhsT operands and no input
or output transposes are needed.  Scores are computed directly transposed
(S^T[k, q]) so the softmax exp on the scalar engine doubles as the
PSUM->SBUF copy and the attention matrix never needs transposing for AV.
Head pairs run concurrently on the PE via disjoint row groups (contract
dim is 64).

Key algebraic tricks (exact, up to float rounding):
  - row_norm^2[q] = Q[q]^T (K^T K) Q[q]: computed via a tiny 64x64 Gram
    matrix instead of reducing the 2048x2048 score matrix.  alpha[q] =
    1/sqrt(t[q]) is folded into Q before the score matmul.  (The
    reference's EPS=1e-12 inside the rescale is negligible: sqrt(t) is
    O(40) for this data distribution; dropped.)
  - softmax denominator: V gets a ones-column appended (M=65 AV matmul),
    so row 64 of the AV accumulator is sum_k exp(s); the division happens
    on the [64, L] attention output, not on the [L, L] attention matrix.
Matmuls run in float32r (full PE column rate; ~1e-4 relative accuracy,
verified on hardware) with fp32 PSUM accumulation.

SBUF note: tile tags are deliberately shared across phases (e.g. LN/ELU
temporaries reuse the K-transpose and Q-tilde slots) to stay inside the
192KB/partition budget; PSUM uses exactly four 2-bank tags.
"""

import numpy as np

import concourse.bacc as bacc
import concourse.bass as bass
import concourse.mybir as mybir
import concourse.tile as tile
from concourse.bass_utils import run_bass_kernel_spmd
from concourse.masks import make_identity

B, C, L, H, HD = 8, 256, 2048, 4, 64
P = 128
NCORES = 8
LH = L // 2  # 1024, attention q-half width
FP32 = mybir.dt.float32
FP32R = mybir.dt.float32r
AF = mybir.ActivationFunctionType
OP = mybir.AluOpType

W_NAMES = ["q_w1", "q_w2", "k_w1", "k_w2", "v_w1", "v_w2", "o_w"]
B_NAMES = ["q_b1", "q_b2", "k_b1", "k_b2", "v_b1", "v_b2", "o_b", "ln_g", "ln_b"]

LN_EPS = 1e-5
INV_C = 1.0 / C
BF16 = mybir.dt.bfloat16
# score/AV matmul dtype: bf16 runs the PE at full rate with fast weight
# load and allows N=1024 moving operands (half the instruction count).
SDT = BF16
NMM = 512

PS_TAGS = ["pA0", "pA1", "pB0", "pB1"]


class Ctx:
    """Holds pools + round-robin psum tag allocation."""

    def __init__(self, nc, tc, pools):
        self.nc = nc
        self.tc = tc
        (self.consts, self.wpool, self.stage, self.gelu, self.qkv, self.attp,
         self.ps, self.rowp, self.bcp, self.dramp) = pools
        self._ps_i = 0

    def ps_tile(self, shape, name):
        tag = PS_TAGS[self._ps_i % 4]
        self._ps_i += 1
        return self.ps.tile(shape, FP32, tag=tag, name=name, bufs=1)


def _linear_T(cx, w_sb, rhs_tiles, out_tiles, act_fn, bias_sb):
    """out^T[m, l] = act(sum_k w[k, m] * rhs^T[k, l] + bias[m]).

    w_sb [P, 2, C] fp32r; rhs_tiles: 2 tiles [P, L] fp32r (contraction
    k-outer); out_tiles: 2 tiles [P, L].  PSUM in [P, LH] chunks.
    """
    nc = cx.nc
    for m in range(2):
        for lh in range(2):
            acc = cx.ps_tile([P, LH], f"lin_acc_{m}_{lh}")
            for lg in range(2):
                psl = slice(lg * 512, (lg + 1) * 512)
                gsl = slice(lh * LH + lg * 512, lh * LH + (lg + 1) * 512)
                for k in range(2):
                    nc.tensor.matmul(
                        acc[:, psl],
                        w_sb[:, k, m * P : (m + 1) * P],
                        rhs_tiles[k][:, gsl],
                        start=(k == 0),
                        stop=(k == 1),
                    )
            osl = slice(lh * LH, (lh + 1) * LH)
            if act_fn is not None:
                cx.nc.scalar.activation(
                    out_tiles[m][:, osl], acc[:], act_fn,
                    bias=bias_sb[:, m : m + 1],
                )
            else:
                nc.vector.tensor_scalar_add(
                    out_tiles[m][:, osl], acc[:], bias_sb[:, m : m + 1]
                )


def _bcast(cx, out_ap, row_ap, name):
    """Broadcast a [1, N] SBUF row across partitions via a DRAM bounce
    (SBUF->SBUF DMA cannot have a zero partition step, DRAM->SBUF can)."""
    nc = cx.nc
    n = row_ap.shape[-1]
    d = cx.dramp.tile([1, n], FP32, tag="drow", name=name, bufs=2)
    nc.sync.dma_start(out=d[:], in_=row_ap)
    nc.sync.dma_start(out=out_ap, in_=d.to_broadcast(list(out_ap.shape)))


def _build_body(cx, phases=4):
    nc = cx.nc
    x_in, out = nc.x_in_t, nc.out_t

    def _dump(tiles):
        for m, t in enumerate(tiles):
            nc.sync.dma_start(out[m * P : (m + 1) * P, :], t.bitcast(FP32)[:])

    # ---- constants (fp32r written via compute-engine rounding copies) ----
    ident_st = cx.consts.tile([P, P], FP32)
    make_identity(nc, ident_st)
    ident = cx.consts.tile([P, P], FP32R)
    nc.vector.tensor_copy(ident[:], ident_st[:])
    ones_st = cx.consts.tile([P, 32], FP32)
    nc.vector.memset(ones_st[:], 1.0)
    ones_64_1 = cx.consts.tile([64, 1], FP32R)
    nc.vector.tensor_copy(ones_64_1[:], ones_st[0:64, 0:1])
    invc_st = cx.consts.tile([P, 1], FP32)
    nc.vector.memset(invc_st[:], INV_C)
    invc_128 = cx.consts.tile([P, 1], FP32R)
    nc.vector.tensor_copy(invc_128[:], invc_st[:])
    eps_sb = cx.consts.tile([1, 1], FP32)
    nc.vector.memset(eps_sb[:], LN_EPS)

    # ---- x load + round ----
    x_re = x_in.rearrange("(ko ki) l -> ki ko l", ki=P)
    xst = cx.stage.tile([P, 2, L], FP32, tag="x_st")
    xr = cx.stage.tile([P, 2, L], FP32R, tag="xr")
    for ko in range(2):
        for xh in range(2):
            xsl = slice(xh * LH, (xh + 1) * LH)
            nc.sync.dma_start(xst[:, ko, xsl], x_re[:, ko, xsl])
            nc.vector.tensor_copy(xr[:, ko, xsl], xst[:, ko, xsl])

    b_sb = {}
    for name in B_NAMES:
        t = cx.consts.tile([P, 2], FP32, name="b_" + name)
        nc.sync.dma_start(
            t[:], getattr(nc, name + "_t").rearrange("(mo mi) -> mi mo", mi=P)
        )
        b_sb[name] = t


    def load_w(name, tag):
        st = cx.wpool.tile([P, 2, C], FP32, tag="w_stage", bufs=2,
                           name=f"wst_{name}")
        nc.sync.dma_start(
            st[:], getattr(nc, name + "_t").rearrange("(ko ki) m -> ki ko m", ki=P)
        )
        wr = cx.wpool.tile([P, 2, C], FP32R, tag=tag, name=f"w_{name}", bufs=1)
        nc.vector.tensor_copy(wr[:], st[:])
        return wr

    # ---- QKV projections ----
    proj = {}
    for p in ("q", "k", "v"):
        w1 = load_w(p + "_w1", "wA")
        w2 = load_w(p + "_w2", "wB")
        g = [cx.gelu.tile([P, L], FP32R, tag=f"g{m}", name=f"g_{p}{m}", bufs=1)
             for m in range(2)]
        _linear_T(cx, w1, [xr[:, 0], xr[:, 1]], g, AF.Gelu, b_sb[p + "_b1"])
        o = [cx.qkv.tile([P, L], FP32R, tag=f"{p}{m}", name=f"{p}_sb{m}", bufs=1)
             for m in range(2)]
        _linear_T(cx, w2, g, o, None, b_sb[p + "_b2"])
        proj[p] = o
    q_sb, k_sb, v_sb = proj["q"], proj["k"], proj["v"]
    if phases == 1:
        _dump(q_sb)
        return
    ow_sb = load_w("o_w", "w_ow")

    # ---- PE transposes: K -> k_t (for Gram), V -> v_t (ones-augmented) ----
    k_t, v_t = [], []
    for ct in range(2):
        kt_tile = cx.attp.tile([P, 16, P], FP32R, tag=f"k_t{ct}",
                               name=f"k_t{ct}", bufs=1)
        vt_tile = cx.attp.tile([P, 16, 130], SDT, tag=f"v_t{ct}",
                               name=f"v_t{ct}", bufs=1)
        nc.vector.tensor_copy(
            vt_tile.rearrange("p l (h c) -> p l h c", h=2)[:, :, :, 64:65],
            ones_st.rearrange("p (l h c) -> p l h c", l=16, h=2),
        )
        for lt0 in range(0, 16, 4):
            trk = cx.ps_tile([P, 512], f"trk_{ct}_{lt0}")
            for j in range(4):
                nc.tensor.transpose(
                    trk.bitcast(FP32R)[:, j * P : (j + 1) * P],
                    k_sb[ct][:, (lt0 + j) * P : (lt0 + j + 1) * P],
                    ident[:],
                )
            nc.vector.tensor_copy(
                kt_tile[:, lt0 : lt0 + 4, :],
                trk.bitcast(FP32R).rearrange("p (l c) -> p l c", l=4),
            )
            trv = cx.ps_tile([P, 512], f"trv_{ct}_{lt0}")
            for j in range(4):
                nc.tensor.transpose(
                    trv.bitcast(FP32R)[:, j * P : (j + 1) * P],
                    v_sb[ct][:, (lt0 + j) * P : (lt0 + j + 1) * P],
                    ident[:],
                )
            nc.vector.tensor_copy(
                vt_tile[:, lt0 : lt0 + 4, :]
                .rearrange("p l (h c) -> p l h c", h=2)[:, :, :, 0:64],
                trv.bitcast(FP32R).rearrange("p (l h c) -> p l h c", l=4, h=2),
            )
        k_t.append(kt_tile)
        v_t.append(vt_tile)

    # ---- Gram matrices -> alpha -> Q-tilde ----
    # Per channel-tile (head pair): one full 128x128 Gram matmul series
    # (the off-diagonal cross-head blocks are discarded), then a
    # block-diagonal G so GQ/QGQ/t/ln/exp/broadcast all run pair-wide:
    #   t[h, q] = sum_d Q_h[d, q] * (G_h Q_h)[d, q],  alpha = exp(-0.5 ln t)
    sel_st = cx.consts.tile([P, 2], FP32)
    nc.vector.memset(sel_st[:], 0.0)
    nc.vector.memset(sel_st[0:64, 0:1], 1.0)
    nc.vector.memset(sel_st[64:128, 1:2], 1.0)
    sel2 = cx.consts.tile([P, 2], FP32R)
    nc.vector.tensor_copy(sel2[:], sel_st[:])
    gz_st = cx.consts.tile([P, P], FP32)
    nc.vector.memset(gz_st[:], 0.0)

    qt_sb = []
    for ct in range(2):
        abc = cx.bcp.tile([P, L], FP32, tag="bc", name=f"abc{ct}", bufs=2)
        g_ps = cx.ps_tile([P, P], f"g_ps{ct}")
        for kt in range(16):
            nc.tensor.matmul(
                g_ps[:], k_t[ct][:, kt, :], k_t[ct][:, kt, :],
                start=(kt == 0), stop=(kt == 15),
            )
        g_pair = cx.rowp.tile([P, P], FP32R, tag="gram", name=f"g_pair{ct}",
                              bufs=1)
        nc.vector.tensor_copy(g_pair[:], gz_st[:])
        for ho in range(2):
            hsl = slice(64 * ho, 64 * ho + 64)
            nc.vector.tensor_copy(g_pair[hsl, hsl], g_ps.bitcast(FP32R)[hsl, hsl])
        qgq = cx.rowp.tile([P, L], FP32R, tag="qgq", name=f"qgq{ct}", bufs=1)
        for lh in range(2):
            lsl = slice(lh * LH, (lh + 1) * LH)
            gq_ps = cx.ps_tile([P, LH], f"gq_ps{ct}{lh}")
            for lg in range(2):
                psl = slice(lg * 512, (lg + 1) * 512)
                gsl = slice(lh * LH + lg * 512, lh * LH + (lg + 1) * 512)
                nc.tensor.matmul(gq_ps[:, psl], g_pair[:], q_sb[ct][:, gsl],
                                 start=True, stop=True)
            nc.vector.tensor_tensor(out=qgq[:, lsl], in0=q_sb[ct][:, lsl],
                                    in1=gq_ps[:], op=OP.mult)
        lnt = cx.rowp.tile([2, L], FP32, tag="rowA", name=f"lnt{ct}", bufs=1)
        for lh in range(2):
            lsl = slice(lh * LH, (lh + 1) * LH)
            t_ps = cx.ps_tile([2, LH], f"t_ps{ct}{lh}")
            for lg in range(2):
                psl = slice(lg * 512, (lg + 1) * 512)
                nc.tensor.matmul(t_ps[:, psl], sel2[:],
                                 qgq[:, lh * LH + lg * 512 : lh * LH + (lg + 1) * 512],
                                 start=True, stop=True)
            nc.scalar.activation(lnt[:, lsl], t_ps[:], AF.Ln)
        a_pair = cx.rowp.tile([2, L], FP32, tag="rowB", name=f"apair{ct}",
                              bufs=1)
        nc.scalar.activation(a_pair[:], lnt[:], AF.Exp, scale=-0.5)
        ad = cx.dramp.tile([2, L], FP32, tag="drow2", name=f"ad{ct}", bufs=2)
        nc.sync.dma_start(out=ad[:], in_=a_pair[:])
        abc_src = bass.AP(tensor=ad.tensor, offset=ad.offset,
                          ap=[[L, 2], [0, 64], [1, L]])
        nc.sync.dma_start(out=abc[:], in_=abc_src)
        qt = cx.gelu.tile([P, L], SDT, tag=f"g{ct}", name=f"qt{ct}", bufs=1)
        nc.vector.tensor_tensor(out=qt[:], in0=q_sb[ct][:], in1=abc[:],
                                op=OP.mult)
        qt_sb.append(qt)
    kb = cx.qkv.tile([P, 2, L], SDT, tag="kb", name="kb", bufs=1)
    for ct in range(2):
        nc.vector.tensor_copy(kb[:, ct, :], k_sb[ct][:])
    if phases == 2:
        _dump(qt_sb)
        return

    # ---- attention + per-half tails ----
    # Emission order matters: engine queues are in-order, so the qh0 tail is
    # emitted after the first qh1 quarter (its PSUM deps are then already
    # satisfied) and tail PSUM tiles use the a-tags, keeping the b-tags
    # attention-only.
    y_sb = [cx.qkv.tile([P, L], FP32R, tag=f"v{ct}", name=f"y{ct}", bufs=1)
            for ct in range(2)]
    z_sb = [cx.qkv.tile([P, L], FP32R, tag=f"q{m}", name=f"z{m}", bufs=1)
            for m in range(2)]

    def quarter(qh, ct, ho):
        q0 = qh * LH
        hslice = slice(q0, q0 + LH)
        hsl = slice(64 * ho, 64 * ho + 64)
        b_ps = cx.ps.tile([65, LH], FP32, tag=PS_TAGS[2 + ho],
                          name=f"av{ct}{qh}{ho}", bufs=1)

        def s_mm(kt):
            a = cx.ps.tile([P, LH], FP32, tag=PS_TAGS[kt % 2],
                           name=f"s{ct}{qh}{kt}{ho}", bufs=1)
            for lg in range(LH // NMM):
                psl = slice(lg * NMM, (lg + 1) * NMM)
                nc.tensor.matmul(
                    a[:, psl],
                    kb[hsl, ct, kt * P : (kt + 1) * P],
                    qt_sb[ct][hsl, q0 + lg * NMM : q0 + (lg + 1) * NMM],
                    start=True, stop=True,
                )
            return a

        # two-deep software pipeline: emit exp[kt], S[kt+1], AV[kt-1] - by
        # the time the in-order PE queue reaches AV[kt-1], exp[kt-1] is long
        # done, so neither S nor AV ever stalls the PE stream
        def av_mm(kt, attn):
            for lg in range(LH // NMM):
                psl = slice(lg * NMM, (lg + 1) * NMM)
                nc.tensor.matmul(
                    b_ps[:, psl],
                    v_t[ct][:, kt, 65 * ho : 65 * ho + 65],
                    attn[:, psl],
                    start=(kt == 0), stop=(kt == 15),
                )

        import os as _os
        _abl = _os.environ.get("ATTN_ABLATE", "")
        a_cur = s_mm(0)
        attn_prev = None
        for kt in range(16):
            attn = cx.attp.tile([P, LH], SDT, tag=f"attn{kt % 2}",
                                name=f"at{ct}{qh}{kt}{ho}", bufs=2)
            if _abl == "exp_half":
                nc.scalar.activation(attn[:, 0:512], a_cur[:, 0:512], AF.Exp)
            else:
                nc.scalar.activation(attn[:], a_cur[:], AF.Exp)
            if kt < 15:
                a_cur = s_mm(kt + 1)
            if attn_prev is not None:
                av_mm(kt - 1, attn_prev)
            attn_prev = attn
        av_mm(15, attn_prev)
        # drain: pull Y/d out of PSUM promptly, then divide
        invd = cx.rowp.tile([1, LH], FP32, tag="rowA",
                            name=f"invd{ct}{qh}{ho}", bufs=1)
        nc.vector.reciprocal(invd[:], b_ps[64:65, :])
        yc = cx.rowp.tile([64, LH], FP32,
                          tag=("qgq" if ho == 0 else "rowC"),
                          name=f"yc{qh}{ct}{ho}", bufs=1)
        nc.vector.tensor_copy(yc[:], b_ps[0:64, :])
        dbc = cx.bcp.tile([64, LH], FP32, tag="bc",
                          name=f"dbc{ct}{qh}{ho}", bufs=2)
        _bcast(cx, dbc[:], invd[:], f"invd_d{ct}{qh}{ho}")
        nc.vector.tensor_tensor(
            out=y_sb[ct][hsl, hslice], in0=yc[:], in1=dbc[:], op=OP.mult,
        )

    def half_tail(qh):
        q0 = qh * LH
        hslice = slice(q0, q0 + LH)
        for m in range(2):
            acc = cx.ps.tile([P, LH], FP32, tag=PS_TAGS[2 + m],
                             name=f"zacc{qh}{m}", bufs=1)
            for lg in range(2):
                psl = slice(lg * 512, (lg + 1) * 512)
                gsl = slice(q0 + lg * 512, q0 + (lg + 1) * 512)
                for ct in range(2):
                    nc.tensor.matmul(
                        acc[:, psl],
                        ow_sb[:, ct, m * P : (m + 1) * P],
                        y_sb[ct][:, gsl],
                        start=(ct == 0), stop=(ct == 1),
                    )
            nc.vector.tensor_scalar_add(z_sb[m][:, hslice], acc[:],
                                        b_sb["o_b"][:, m : m + 1])
        z2h = [cx.rowp.tile([P, LH], FP32R,
                            tag=("qgq" if m == 0 else "z2b"),
                            name=f"z2_{qh}{m}", bufs=1) for m in range(2)]
        for m in range(2):
            nc.vector.tensor_tensor(out=z2h[m][:], in0=z_sb[m][:, hslice],
                                    in1=z_sb[m][:, hslice], op=OP.mult)
        s1_ps = cx.ps.tile([1, LH], FP32, tag=PS_TAGS[2], name=f"s1_{qh}",
                           bufs=1)
        s2_ps = cx.ps.tile([1, LH], FP32, tag=PS_TAGS[3], name=f"s2_{qh}",
                           bufs=1)
        for lg in range(2):
            psl = slice(lg * 512, (lg + 1) * 512)
            gsl = slice(q0 + lg * 512, q0 + (lg + 1) * 512)
            for m in range(2):
                nc.tensor.matmul(s1_ps[:, psl], invc_128[:], z_sb[m][:, gsl],
                                 start=(m == 0), stop=(m == 1))
            for m in range(2):
                nc.tensor.matmul(s2_ps[:, psl], invc_128[:], z2h[m][:, psl],
                                 start=(m == 0), stop=(m == 1))
        # mean/var row chain ([1, LH] vectors)
        mu = cx.rowp.tile([1, LH], FP32, tag="rowA", name=f"mu{qh}", bufs=1)
        nc.vector.tensor_copy(mu[:], s1_ps[:])
        var = cx.rowp.tile([1, LH], FP32, tag="rowB", name=f"var{qh}", bufs=1)
        nc.vector.tensor_tensor(out=var[:], in0=mu[:], in1=mu[:], op=OP.mult)
        var2 = cx.rowp.tile([1, LH], FP32, tag="rowC", name=f"var2{qh}",
                            bufs=1)
        nc.vector.scalar_tensor_tensor(out=var2[:], in0=s2_ps[:], scalar=0.0,
                                       in1=var[:], op0=OP.add,
                                       op1=OP.subtract)
        lnv = cx.rowp.tile([1, LH], FP32, tag="rowB", name=f"lnv{qh}", bufs=1)
        nc.scalar.activation(lnv[:], var2[:], AF.Ln, bias=eps_sb[:])
        rstd = cx.rowp.tile([1, LH], FP32, tag="rowC", name=f"rstd{qh}",
                            bufs=1)
        nc.scalar.activation(rstd[:], lnv[:], AF.Exp, scale=-0.5)
        mrs = cx.rowp.tile([1, LH], FP32, tag="rowB", name=f"mrs{qh}", bufs=1)
        nc.vector.tensor_tensor(out=mrs[:], in0=mu[:], in1=rstd[:],
                                op=OP.mult)
        ab = cx.stage.tile([P, 2, LH], FP32, tag="x_st", name=f"ab{qh}",
                           bufs=1)
        abd = cx.dramp.tile([2, LH], FP32, tag="drow2", name=f"abd{qh}",
                            bufs=2)
        nc.sync.dma_start(out=abd[0:1, :], in_=rstd[:])
        nc.sync.dma_start(out=abd[1:2, :], in_=mrs[:])
        ab_src = bass.AP(tensor=abd.tensor, offset=abd.offset,
                         ap=[[0, P], [LH, 2], [1, LH]])
        nc.sync.dma_start(out=ab[:], in_=ab_src)
        for m in range(2):
            # reuse the already-consumed y_sb region of this l-half as the
            # LN/ELU workspace (write-after-read; no extra SBUF slot)
            u = y_sb[m][:, hslice]
            nc.vector.tensor_tensor(out=u[:], in0=z_sb[m][:, hslice],
                                    in1=ab[:, 0, :], op=OP.mult)
            nc.vector.tensor_tensor(out=u[:], in0=u[:], in1=ab[:, 1, :],
                                    op=OP.subtract)
            nc.vector.tensor_scalar(
                out=u[:], in0=u[:],
                scalar1=b_sb["ln_g"][:, m : m + 1],
                scalar2=b_sb["ln_b"][:, m : m + 1],
                op0=OP.mult, op1=OP.add,
            )
            # elu(u) + x = relu(u) + exp(min(u,0)) - 1 + x
            neg = cx.attp.tile([P, LH], FP32, tag="k_t0", name=f"neg{qh}{m}",
                               bufs=1)
            nc.vector.tensor_scalar_min(neg[:], u[:], 0.0)
            e = cx.attp.tile([P, LH], FP32, tag="k_t1", name=f"e{qh}{m}",
                             bufs=1)
            nc.scalar.activation(e[:], neg[:], AF.Exp)
            nc.vector.scalar_tensor_tensor(out=u[:], in0=u[:], scalar=0.0,
                                           in1=e[:], op0=OP.max, op1=OP.add)
            nc.vector.scalar_tensor_tensor(out=u[:], in0=u[:], scalar=-1.0,
                                           in1=xr[:, m, hslice], op0=OP.add,
                                           op1=OP.add)
            nc.sync.dma_start(out[m * P : (m + 1) * P, hslice],
                              u.bitcast(FP32)[:])

    if phases == 3:
        for qh in range(2):
            for ct in range(2):
                for ho in range(2):
                    quarter(qh, ct, ho)
        _dump(y_sb)
        return
    for ct in range(2):
        for ho in range(2):
            quarter(0, ct, ho)
    half_tail(0)
    for ct in range(2):
        for ho in range(2):
            quarter(1, ct, ho)
    half_tail(1)

def _steer_act_tables():
    """The act-table-load pass picks the first set containing each
    function, which thrashes natural_log <-> exp_and_others when a kernel
    uses both Ln and Exp.  Empty out the single-function sets so both
    resolve to natural_log_exp_and_others (ids keep their positions)."""
    import concourse.hw_specs as hw_specs
    if getattr(hw_specs, "_act_tables_steered", False):
        return
    orig = hw_specs.get_activation_tables

    def patched(arch):
        t = dict(orig(arch))
        for k in ("natural_log", "exp_and_others", "exp_and_friends"):
            if k in t:
                t[k] = set()
        return t

    hw_specs.get_activation_tables = patched
    bacc.get_activation_tables = patched
    hw_specs._act_tables_steered = True


def build_nc(repeat: int = 1, phases: int = 4):
    _steer_act_tables()
    nc = bacc.Bacc("TRN2", target_bir_lowering=False)
    nc.x_in_t = nc.dram_tensor("x_in", [C, L], FP32, kind="ExternalInput")
    for name in W_NAMES:
        setattr(nc, name + "_t",
                nc.dram_tensor(name, [C, C], FP32, kind="ExternalInput"))
    for name in B_NAMES:
        setattr(nc, name + "_t",
                nc.dram_tensor(name, [C], FP32, kind="ExternalInput"))
    nc.out_t = nc.dram_tensor("out", [C, L], FP32, kind="ExternalOutput")

    with tile.TileContext(nc) as tc:
        with (
            tc.tile_pool(name="consts", bufs=1) as consts,
            tc.tile_pool(name="wpool", bufs=1) as wpool,
            tc.tile_pool(name="stage", bufs=1) as stage,
            tc.tile_pool(name="gelu", bufs=1) as gelu,
            tc.tile_pool(name="qkv", bufs=1) as qkv,
            tc.tile_pool(name="attp", bufs=1) as attp,
            tc.tile_pool(name="ps", bufs=1, space="PSUM") as ps,
            tc.tile_pool(name="rowp", bufs=1) as rowp,
            tc.tile_pool(name="bcp", bufs=1) as bcp,
            tc.tile_pool(name="dramp", bufs=2, space="DRAM") as dramp,
        ):
            pools = (consts, wpool, stage, gelu, qkv, attp, ps, rowp, bcp,
                     dramp)
            cx = Ctx(nc, tc, pools)
            if repeat == 1:
                _build_body(cx, phases)
            else:
                with tc.For_i(0, repeat, 1):
                    _build_body(cx, phases)
    nc.finalize()
    return nc


_NC_CACHE = {}


def _get_nc(repeat=1, phases=4):
    key = (repeat, phases)
    if key not in _NC_CACHE:
        _NC_CACHE[key] = build_nc(repeat, phases)
    return _NC_CACHE[key]


def kernel(**inputs: np.ndarray) -> np.ndarray:
    nc = _get_nc()
    x_in = np.ascontiguousarray(inputs["x_in"], dtype=np.float32)
    shared = {}
    for name in W_NAMES + B_NAMES:
        shared[name] = np.ascontiguousarray(inputs[name], dtype=np.float32)
    in_maps = [dict(shared, x_in=x_in[b]) for b in range(NCORES)]
    res = run_bass_kernel_spmd(nc, in_maps, core_ids=list(range(NCORES)))
    return np.stack([res.results[b]["out"] for b in range(NCORES)], axis=0)


# revision 41
# speedup vs baseline: 1.0047x; 1.0047x over previous
"""Trainium2 Bass kernel for nn_DilatedAttentionBlock_attention.

Per-core work (data-parallel over batch, 8 cores):
  x [C=256, L=2048] -> QKV MLPs -> 4-head attention with Lipschitz score
  rescale -> out-proj -> LayerNorm -> ELU + residual -> out [C, L].

Everything stays in channel-major ("transposed") [C, L] layout, which is the
native layout of x_in, so weights act as natural lhsT operands and no input
or output transposes are needed.  Scores are computed directly transposed
(S^T[k, q]) so the softmax exp on the scalar engine doubles as the
PSUM->SBUF copy and the attention matrix never needs transposing for AV.
Head pairs run concurrently on the PE via disjoint row groups (contract
dim is 64).

Key algebraic tricks (exact, up to float rounding):
  - row_norm^2[q] = Q[q]^T (K^T K) Q[q]: computed via a tiny 64x64 Gram
    matrix instead of reducing the 2048x2048 score matrix.  alpha[q] =
    1/sqrt(t[q]) is folded into Q before the score matmul.  (The
    reference's EPS=1e-12 inside the rescale is negligible: sqrt(t) is
    O(40) for this data distribution; dropped.)
  - softmax denominator: V gets a ones-column appended (M=65 AV matmul),
    so row 64 of the AV accumulator is sum_k exp(s); the division happens
    on the [64, L] attention output, not on the [L, L] attention matrix.
Matmuls run in float32r (full PE column rate; ~1e-4 relative accuracy,
verified on hardware) with fp32 PSUM accumulation.

SBUF note: tile tags are deliberately shared across phases (e.g. LN/ELU
temporaries reuse the K-transpose and Q-tilde slots) to stay inside the
192KB/partition budget; PSUM uses exactly four 2-bank tags.
"""

import numpy as np

import concourse.bacc as bacc
import concourse.bass as bass
import concourse.mybir as mybir
import concourse.tile as tile
from concourse.bass_utils import run_bass_kernel_spmd
from concourse.masks import make_identity

B, C, L, H, HD = 8, 256, 2048, 4, 64
P = 128
NCORES = 8
LH = L // 2  # 1024, attention q-half width
FP32 = mybir.dt.float32
FP32R = mybir.dt.float32r
AF = mybir.ActivationFunctionType
OP = mybir.AluOpType

W_NAMES = ["q_w1", "q_w2", "k_w1", "k_w2", "v_w1", "v_w2", "o_w"]
B_NAMES = ["q_b1", "q_b2", "k_b1", "k_b2", "v_b1", "v_b2", "o_b", "ln_g", "ln_b"]

LN_EPS = 1e-5
INV_C = 1.0 / C
BF16 = mybir.dt.bfloat16
# score/AV matmul dtype: bf16 runs the PE at full rate with fast weight
# load and allows N=1024 moving operands (half the instruction count).
SDT = BF16
NMM = 512

PS_TAGS = ["pA0", "pA1", "pB0", "pB1"]


class Ctx:
    """Holds pools + round-robin psum tag allocation."""

    def __init__(self, nc, tc, pools):
        self.nc = nc
        self.tc = tc
        (self.consts, self.wpool, self.stage, self.gelu, self.qkv, self.attp,
         self.ps, self.rowp, self.bcp, self.dramp) = pools
        self._ps_i = 0

    def ps_tile(self, shape, name):
        tag = PS_TAGS[self._ps_i % 4]
        self._ps_i += 1
        return self.ps.tile(shape, FP32, tag=tag, name=name, bufs=1)


def _linear_T(cx, w_sb, rhs_tiles, out_tiles, act_fn, bias_sb):
    """out^T[m, l] = act(sum_k w[k, m] * rhs^T[k, l] + bias[m]).

    w_sb [P, 2, C] fp32r; rhs_tiles: 2 tiles [P, L] fp32r (contraction
    k-outer); out_tiles: 2 tiles [P, L].  PSUM in [P, LH] chunks.
    """
    nc = cx.nc
    for m in range(2):
        for lh in range(2):
            acc = cx.ps_tile([P, LH], f"lin_acc_{m}_{lh}")
            for lg in range(2):
                psl = slice(lg * 512, (lg + 1) * 512)
                gsl = slice(lh * LH + lg * 512, lh * LH + (lg + 1) * 512)
                for k in range(2):
                    nc.tensor.matmul(
                        acc[:, psl],
                        w_sb[:, k, m * P : (m + 1) * P],
                        rhs_tiles[k][:, gsl],
                        start=(k == 0),
                        stop=(k == 1),
                    )
            osl = slice(lh * LH, (lh + 1) * LH)
            if act_fn is not None:
                cx.nc.scalar.activation(
                    out_tiles[m][:, osl], acc[:], act_fn,
                    bias=bias_sb[:, m : m + 1],
                )
            else:
                nc.vector.tensor_scalar_add(
                    out_tiles[m][:, osl], acc[:], bias_sb[:, m : m + 1]
                )


def _bcast(cx, out_ap, row_ap, name):
    """Broadcast a [1, N] SBUF row across partitions via a DRAM bounce
    (SBUF->SBUF DMA cannot have a zero partition step, DRAM->SBUF can)."""
    nc = cx.nc
    n = row_ap.shape[-1]
    d = cx.dramp.tile([1, n], FP32, tag="drow", name=name, bufs=2)
    nc.sync.dma_start(out=d[:], in_=row_ap)
    nc.sync.dma_start(out=out_ap, in_=d.to_broadcast(list(out_ap.shape)))


def _build_body(cx, phases=4):
    nc = cx.nc
    x_in, out = nc.x_in_t, nc.out_t

    def _dump(tiles):
        for m, t in enumerate(tiles):
            v = t.bitcast(FP32)
            nc.sync.dma_start(out[m * P : (m + 1) * P, 0 : v.shape[-1]], v[:])

    # ---- constants (fp32r written via compute-engine rounding copies) ----
    ident_st = cx.consts.tile([P, P], FP32)
    make_identity(nc, ident_st)
    ident = cx.consts.tile([P, P], FP32R)
    nc.vector.tensor_copy(ident[:], ident_st[:])
    ones_st = cx.consts.tile([P, 32], FP32)
    nc.vector.memset(ones_st[:], 1.0)
    ones_64_1 = cx.consts.tile([64, 1], FP32R)
    nc.vector.tensor_copy(ones_64_1[:], ones_st[0:64, 0:1])
    invc_st = cx.consts.tile([P, 1], FP32)
    nc.vector.memset(invc_st[:], INV_C)
    invc_128 = cx.consts.tile([P, 1], FP32R)
    nc.vector.tensor_copy(invc_128[:], invc_st[:])
    eps_sb = cx.consts.tile([1, 1], FP32)
    nc.vector.memset(eps_sb[:], LN_EPS)

    # ---- x load + round ----
    x_re = x_in.rearrange("(ko ki) l -> ki ko l", ki=P)
    xst = cx.stage.tile([P, 2, L], FP32, tag="x_st")
    xr = cx.stage.tile([P, 2, L], FP32R, tag="xr")
    for ko in range(2):
        for xh in range(2):
            xsl = slice(xh * LH, (xh + 1) * LH)
            nc.sync.dma_start(xst[:, ko, xsl], x_re[:, ko, xsl])
            nc.vector.tensor_copy(xr[:, ko, xsl], xst[:, ko, xsl])

    b_sb = {}
    for name in B_NAMES:
        t = cx.consts.tile([P, 2], FP32, name="b_" + name)
        nc.sync.dma_start(
            t[:], getattr(nc, name + "_t").rearrange("(mo mi) -> mi mo", mi=P)
        )
        b_sb[name] = t


    def load_w(name, tag):
        st = cx.wpool.tile([P, 2, C], FP32, tag="w_stage", bufs=2,
                           name=f"wst_{name}")
        nc.sync.dma_start(
            st[:], getattr(nc, name + "_t").rearrange("(ko ki) m -> ki ko m", ki=P)
        )
        wr = cx.wpool.tile([P, 2, C], FP32R, tag=tag, name=f"w_{name}", bufs=1)
        nc.vector.tensor_copy(wr[:], st[:])
        return wr

    # ---- QKV projections ----
    proj = {}
    for p in ("q", "k", "v"):
        w1 = load_w(p + "_w1", "wA")
        w2 = load_w(p + "_w2", "wB")
        g = [cx.gelu.tile([P, L], FP32R, tag=f"g{m}", name=f"g_{p}{m}", bufs=1)
             for m in range(2)]
        _linear_T(cx, w1, [xr[:, 0], xr[:, 1]], g, AF.Gelu, b_sb[p + "_b1"])
        o = [cx.qkv.tile([P, L], FP32R, tag=f"{p}{m}", name=f"{p}_sb{m}", bufs=1)
             for m in range(2)]
        _linear_T(cx, w2, g, o, None, b_sb[p + "_b2"])
        proj[p] = o
    q_sb, k_sb, v_sb = proj["q"], proj["k"], proj["v"]
    if phases == 1:
        _dump(q_sb)
        return
    ow_sb = load_w("o_w", "w_ow")

    # ---- PE transposes: K -> k_t (for Gram), V -> v_t (ones-augmented) ----
    k_t, v_t = [], []
    for ct in range(2):
        kt_tile = cx.attp.tile([P, 16, P], FP32R, tag=f"k_t{ct}",
                               name=f"k_t{ct}", bufs=1)
        vt_tile = cx.attp.tile([P, 16, 130], SDT, tag=f"v_t{ct}",
                               name=f"v_t{ct}", bufs=1)
        nc.vector.tensor_copy(
            vt_tile.rearrange("p l (h c) -> p l h c", h=2)[:, :, :, 64:65],
            ones_st.rearrange("p (l h c) -> p l h c", l=16, h=2),
        )
        for lt0 in range(0, 16, 4):
            trk = cx.ps_tile([P, 512], f"trk_{ct}_{lt0}")
            for j in range(4):
                nc.tensor.transpose(
                    trk.bitcast(FP32R)[:, j * P : (j + 1) * P],
                    k_sb[ct][:, (lt0 + j) * P : (lt0 + j + 1) * P],
                    ident[:],
                )
            nc.vector.tensor_copy(
                kt_tile[:, lt0 : lt0 + 4, :],
                trk.bitcast(FP32R).rearrange("p (l c) -> p l c", l=4),
            )
            trv = cx.ps_tile([P, 512], f"trv_{ct}_{lt0}")
            for j in range(4):
                nc.tensor.transpose(
                    trv.bitcast(FP32R)[:, j * P : (j + 1) * P],
                    v_sb[ct][:, (lt0 + j) * P : (lt0 + j + 1) * P],
                    ident[:],
                )
            nc.vector.tensor_copy(
                vt_tile[:, lt0 : lt0 + 4, :]
                .rearrange("p l (h c) -> p l h c", h=2)[:, :, :, 0:64],
                trv.bitcast(FP32R).rearrange("p (l h c) -> p l h c", l=4, h=2),
            )
        k_t.append(kt_tile)
        v_t.append(vt_tile)

    # ---- Gram matrices -> alpha -> Q-tilde ----
    # Per channel-tile (head pair): one full 128x128 Gram matmul series
    # (the off-diagonal cross-head blocks are discarded), then a
    # block-diagonal G so GQ/QGQ/t/ln/exp/broadcast all run pair-wide:
    #   t[h, q] = sum_d Q_h[d, q] * (G_h Q_h)[d, q],  alpha = exp(-0.5 ln t)
    sel_st = cx.consts.tile([P, 2], FP32)
    nc.vector.memset(sel_st[:], 0.0)
    nc.vector.memset(sel_st[0:64, 0:1], 1.0)
    nc.vector.memset(sel_st[64:128, 1:2], 1.0)
    sel2 = cx.consts.tile([P, 2], FP32R)
    nc.vector.tensor_copy(sel2[:], sel_st[:])
    gz_st = cx.consts.tile([P, P], FP32)
    nc.vector.memset(gz_st[:], 0.0)

    qt_sb = []
    for ct in range(2):
        abc = cx.bcp.tile([P, L], FP32, tag="bc", name=f"abc{ct}", bufs=2)
        g_ps = cx.ps_tile([P, P], f"g_ps{ct}")
        for kt in range(16):
            nc.tensor.matmul(
                g_ps[:], k_t[ct][:, kt, :], k_t[ct][:, kt, :],
                start=(kt == 0), stop=(kt == 15),
            )
        g_pair = cx.rowp.tile([P, P], FP32R, tag="gram", name=f"g_pair{ct}",
                              bufs=1)
        nc.vector.tensor_copy(g_pair[:], gz_st[:])
        for ho in range(2):
            hsl = slice(64 * ho, 64 * ho + 64)
            nc.vector.tensor_copy(g_pair[hsl, hsl], g_ps.bitcast(FP32R)[hsl, hsl])
        qgq = cx.rowp.tile([P, L], FP32R, tag="qgq", name=f"qgq{ct}", bufs=1)
        for lh in range(2):
            lsl = slice(lh * LH, (lh + 1) * LH)
            gq_ps = cx.ps_tile([P, LH], f"gq_ps{ct}{lh}")
            for lg in range(2):
                psl = slice(lg * 512, (lg + 1) * 512)
                gsl = slice(lh * LH + lg * 512, lh * LH + (lg + 1) * 512)
                nc.tensor.matmul(gq_ps[:, psl], g_pair[:], q_sb[ct][:, gsl],
                                 start=True, stop=True)
            nc.vector.tensor_tensor(out=qgq[:, lsl], in0=q_sb[ct][:, lsl],
                                    in1=gq_ps[:], op=OP.mult)
        lnt = cx.rowp.tile([2, L], FP32, tag="rowA", name=f"lnt{ct}", bufs=1)
        for lh in range(2):
            lsl = slice(lh * LH, (lh + 1) * LH)
            t_ps = cx.ps_tile([2, LH], f"t_ps{ct}{lh}")
            for lg in range(2):
                psl = slice(lg * 512, (lg + 1) * 512)
                nc.tensor.matmul(t_ps[:, psl], sel2[:],
                                 qgq[:, lh * LH + lg * 512 : lh * LH + (lg + 1) * 512],
                                 start=True, stop=True)
            nc.scalar.activation(lnt[:, lsl], t_ps[:], AF.Ln)
        a_pair = cx.rowp.tile([2, L], FP32, tag="rowB", name=f"apair{ct}",
                              bufs=1)
        nc.scalar.activation(a_pair[:], lnt[:], AF.Exp, scale=-0.5)
        ad = cx.dramp.tile([2, L], FP32, tag="drow2", name=f"ad{ct}", bufs=2)
        nc.sync.dma_start(out=ad[:], in_=a_pair[:])
        abc_src = bass.AP(tensor=ad.tensor, offset=ad.offset,
                          ap=[[L, 2], [0, 64], [1, L]])
        nc.sync.dma_start(out=abc[:], in_=abc_src)
        qt = cx.gelu.tile([P, L], SDT, tag=f"g{ct}", name=f"qt{ct}", bufs=1)
        nc.vector.tensor_tensor(out=qt[:], in0=q_sb[ct][:], in1=abc[:],
                                op=OP.mult)
        qt_sb.append(qt)
    kb = cx.qkv.tile([P, 2, L], SDT, tag="kb", name="kb", bufs=1)
    for ct in range(2):
        nc.vector.tensor_copy(kb[:, ct, :], k_sb[ct][:])
    if phases == 2:
        _dump(qt_sb)
        return

    # ---- attention + per-half tails ----
    # Emission order matters: engine queues are in-order, so the qh0 tail is
    # emitted after the first qh1 quarter (its PSUM deps are then already
    # satisfied) and tail PSUM tiles use the a-tags, keeping the b-tags
    # attention-only.
    y_sb = [cx.qkv.tile([P, L], FP32R, tag=f"v{ct}", name=f"y{ct}", bufs=1)
            for ct in range(2)]
    z_sb = [cx.qkv.tile([P, L], FP32R, tag=f"q{m}", name=f"z{m}", bufs=1)
            for m in range(2)]

    def quarter(qh, ct, ho):
        q0 = qh * LH
        hslice = slice(q0, q0 + LH)
        hsl = slice(64 * ho, 64 * ho + 64)
        b_ps = cx.ps.tile([65, LH], FP32, tag=PS_TAGS[2 + ho],
                          name=f"av{ct}{qh}{ho}", bufs=1)

        def s_mm(kt):
            a = cx.ps.tile([P, LH], FP32, tag=PS_TAGS[kt % 2],
                           name=f"s{ct}{qh}{kt}{ho}", bufs=1)
            for lg in range(LH // NMM):
                psl = slice(lg * NMM, (lg + 1) * NMM)
                nc.tensor.matmul(
                    a[:, psl],
                    kb[hsl, ct, kt * P : (kt + 1) * P],
                    qt_sb[ct][hsl, q0 + lg * NMM : q0 + (lg + 1) * NMM],
                    start=True, stop=True,
                )
            return a

        # two-deep software pipeline: emit exp[kt], S[kt+1], AV[kt-1] - by
        # the time the in-order PE queue reaches AV[kt-1], exp[kt-1] is long
        # done, so neither S nor AV ever stalls the PE stream
        def av_mm(kt, attn):
            for lg in range(LH // NMM):
                psl = slice(lg * NMM, (lg + 1) * NMM)
                nc.tensor.matmul(
                    b_ps[:, psl],
                    v_t[ct][:, kt, 65 * ho : 65 * ho + 65],
                    attn[:, psl],
                    start=(kt == 0), stop=(kt == 15),
                )

        import os as _os
        _abl = _os.environ.get("ATTN_ABLATE", "")
        a_cur = s_mm(0)
        attn_prev = None
        for kt in range(16):
            attn = cx.attp.tile([P, LH], SDT, tag=f"attn{kt % 2}",
                                name=f"at{ct}{qh}{kt}{ho}", bufs=3)
            if _abl == "exp_half":
                nc.scalar.activation(attn[:, 0:512], a_cur[:, 0:512], AF.Exp)
            else:
                nc.scalar.activation(attn[:], a_cur[:], AF.Exp)
            if kt < 15:
                a_cur = s_mm(kt + 1)
            if attn_prev is not None:
                av_mm(kt - 1, attn_prev)
            attn_prev = attn
        av_mm(15, attn_prev)
        # drain: pull Y/d out of PSUM promptly, then divide
        invd = cx.rowp.tile([1, LH], FP32, tag="rowA",
                            name=f"invd{ct}{qh}{ho}", bufs=1)
        nc.vector.reciprocal(invd[:], b_ps[64:65, :])
        yc = cx.rowp.tile([64, LH], FP32,
                          tag=("qgq" if ho == 0 else "rowC"),
                          name=f"yc{qh}{ct}{ho}", bufs=1)
        nc.vector.tensor_copy(yc[:], b_ps[0:64, :])
        dbc = cx.bcp.tile([64, LH], FP32, tag="bc",
                          name=f"dbc{ct}{qh}{ho}", bufs=2)
        _bcast(cx, dbc[:], invd[:], f"invd_d{ct}{qh}{ho}")
        nc.vector.tensor_tensor(
            out=y_sb[ct][hsl, hslice], in0=yc[:], in1=dbc[:], op=OP.mult,
        )

    def half_tail(qh):
        q0 = qh * LH
        hslice = slice(q0, q0 + LH)
        for m in range(2):
            acc = cx.ps.tile([P, LH], FP32, tag=PS_TAGS[2 + m],
                             name=f"zacc{qh}{m}", bufs=1)
            for lg in range(2):
                psl = slice(lg * 512, (lg + 1) * 512)
                gsl = slice(q0 + lg * 512, q0 + (lg + 1) * 512)
                for ct in range(2):
                    nc.tensor.matmul(
                        acc[:, psl],
                        ow_sb[:, ct, m * P : (m + 1) * P],
                        y_sb[ct][:, gsl],
                        start=(ct == 0), stop=(ct == 1),
                    )
            nc.vector.tensor_scalar_add(z_sb[m][:, hslice], acc[:],
                                        b_sb["o_b"][:, m : m + 1])
        z2h = [cx.rowp.tile([P, LH], FP32R,
                            tag=("qgq" if m == 0 else "z2b"),
                            name=f"z2_{qh}{m}", bufs=1) for m in range(2)]
        for m in range(2):
            nc.vector.tensor_tensor(out=z2h[m][:], in0=z_sb[m][:, hslice],
                                    in1=z_sb[m][:, hslice], op=OP.mult)
        s1_ps = cx.ps.tile([1, LH], FP32, tag=PS_TAGS[2], name=f"s1_{qh}",
                           bufs=1)
        s2_ps = cx.ps.tile([1, LH], FP32, tag=PS_TAGS[3], name=f"s2_{qh}",
                           bufs=1)
        for lg in range(2):
            psl = slice(lg * 512, (lg + 1) * 512)
            gsl = slice(q0 + lg * 512, q0 + (lg + 1) * 512)
            for m in range(2):
                nc.tensor.matmul(s1_ps[:, psl], invc_128[:], z_sb[m][:, gsl],
                                 start=(m == 0), stop=(m == 1))
            for m in range(2):
                nc.tensor.matmul(s2_ps[:, psl], invc_128[:], z2h[m][:, psl],
                                 start=(m == 0), stop=(m == 1))
        # mean/var row chain ([1, LH] vectors)
        mu = cx.rowp.tile([1, LH], FP32, tag="rowA", name=f"mu{qh}", bufs=1)
        nc.vector.tensor_copy(mu[:], s1_ps[:])
        var = cx.rowp.tile([1, LH], FP32, tag="rowB", name=f"var{qh}", bufs=1)
        nc.vector.tensor_tensor(out=var[:], in0=mu[:], in1=mu[:], op=OP.mult)
        var2 = cx.rowp.tile([1, LH], FP32, tag="rowC", name=f"var2{qh}",
                            bufs=1)
        nc.vector.scalar_tensor_tensor(out=var2[:], in0=s2_ps[:], scalar=0.0,
                                       in1=var[:], op0=OP.add,
                                       op1=OP.subtract)
        lnv = cx.rowp.tile([1, LH], FP32, tag="rowB", name=f"lnv{qh}", bufs=1)
        nc.scalar.activation(lnv[:], var2[:], AF.Ln, bias=eps_sb[:])
        rstd = cx.rowp.tile([1, LH], FP32, tag="rowC", name=f"rstd{qh}",
                            bufs=1)
        nc.scalar.activation(rstd[:], lnv[:], AF.Exp, scale=-0.5)
        mrs = cx.rowp.tile([1, LH], FP32, tag="rowB", name=f"mrs{qh}", bufs=1)
        nc.vector.tensor_tensor(out=mrs[:], in0=mu[:], in1=rstd[:],
                                op=OP.mult)
        ab = cx.stage.tile([P, 2, LH], FP32, tag="x_st", name=f"ab{qh}",
                           bufs=1)
        abd = cx.dramp.tile([2, LH], FP32, tag="drow2", name=f"abd{qh}",
                            bufs=2)
        nc.sync.dma_start(out=abd[0:1, :], in_=rstd[:])
        nc.sync.dma_start(out=abd[1:2, :], in_=mrs[:])
        ab_src = bass.AP(tensor=abd.tensor, offset=abd.offset,
                         ap=[[0, P], [LH, 2], [1, LH]])
        nc.sync.dma_start(out=ab[:], in_=ab_src)
        for m in range(2):
            # reuse the already-consumed y_sb region of this l-half as the
            # LN/ELU workspace (write-after-read; no extra SBUF slot)
            u = y_sb[m][:, hslice]
            nc.vector.tensor_tensor(out=u[:], in0=z_sb[m][:, hslice],
                                    in1=ab[:, 0, :], op=OP.mult)
            nc.vector.tensor_tensor(out=u[:], in0=u[:], in1=ab[:, 1, :],
                                    op=OP.subtract)
            nc.vector.tensor_scalar(
                out=u[:], in0=u[:],
                scalar1=b_sb["ln_g"][:, m : m + 1],
                scalar2=b_sb["ln_b"][:, m : m + 1],
                op0=OP.mult, op1=OP.add,
            )
            # elu(u) + x = relu(u) + exp(min(u,0)) - 1 + x
            neg = cx.attp.tile([P, LH], FP32, tag="k_t0", name=f"neg{qh}{m}",
                               bufs=1)
            nc.vector.tensor_scalar_min(neg[:], u[:], 0.0)
            e = cx.attp.tile([P, LH], FP32, tag="k_t1", name=f"e{qh}{m}",
                             bufs=1)
            nc.scalar.activation(e[:], neg[:], AF.Exp)
            nc.vector.scalar_tensor_tensor(out=u[:], in0=u[:], scalar=0.0,
                                           in1=e[:], op0=OP.max, op1=OP.add)
            nc.vector.scalar_tensor_tensor(out=u[:], in0=u[:], scalar=-1.0,
                                           in1=xr[:, m, hslice], op0=OP.add,
                                           op1=OP.add)
            nc.sync.dma_start(out[m * P : (m + 1) * P, hslice],
                              u.bitcast(FP32)[:])

    if phases == 3:
        for qh in range(2):
            for ct in range(2):
                for ho in range(2):
                    quarter(qh, ct, ho)
        _dump(y_sb)
        return
    for ct in range(2):
        for ho in range(2):
            quarter(0, ct, ho)
    half_tail(0)
    for ct in range(2):
        for ho in range(2):
            quarter(1, ct, ho)
    half_tail(1)

def _steer_act_tables():
    """The act-table-load pass picks the first set containing each
    function, which thrashes natural_log <-> exp_and_others when a kernel
    uses both Ln and Exp.  Empty out the single-function sets so both
    resolve to natural_log_exp_and_others (ids keep their positions)."""
    import concourse.hw_specs as hw_specs
    if getattr(hw_specs, "_act_tables_steered", False):
        return
    orig = hw_specs.get_activation_tables

    def patched(arch):
        t = dict(orig(arch))
        for k in ("natural_log", "exp_and_others", "exp_and_friends"):
            if k in t:
                t[k] = set()
        return t

    hw_specs.get_activation_tables = patched
    bacc.get_activation_tables = patched
    hw_specs._act_tables_steered = True


def build_nc(repeat: int = 1, phases: int = 4):
    _steer_act_tables()
    nc = bacc.Bacc("TRN2", target_bir_lowering=False)
    nc.x_in_t = nc.dram_tensor("x_in", [C, L], FP32, kind="ExternalInput")
    for name in W_NAMES:
        setattr(nc, name + "_t",
                nc.dram_tensor(name, [C, C], FP32, kind="ExternalInput"))
    for name in B_NAMES:
        setattr(nc, name + "_t",
                nc.dram_tensor(name, [C], FP32, kind="ExternalInput"))
    nc.out_t = nc.dram_tensor("out", [C, L], FP32, kind="ExternalOutput")

    with tile.TileContext(nc) as tc:
        with (
            tc.tile_pool(name="consts", bufs=1) as consts,
            tc.tile_pool(name="wpool", bufs=1) as wpool,
            tc.tile_pool(name="stage", bufs=1) as stage,
            tc.tile_pool(name="gelu", bufs=1) as gelu,
            tc.tile_pool(name="qkv", bufs=1) as qkv,
            tc.tile_pool(name="attp", bufs=1) as attp,
            tc.tile_pool(name="ps", bufs=1, space="PSUM") as ps,
            tc.tile_pool(name="rowp", bufs=1) as rowp,
            tc.tile_pool(name="bcp", bufs=1) as bcp,
            tc.tile_pool(name="dramp", bufs=2, space="DRAM") as dramp,
        ):
            pools = (consts, wpool, stage, gelu, qkv, attp, ps, rowp, bcp,
                     dramp)
            cx = Ctx(nc, tc, pools)
            if repeat == 1:
                _build_body(cx, phases)
            else:
                with tc.For_i(0, repeat, 1,
                              hint_engines=(mybir.EngineType.PE,
                                            mybir.EngineType.Activation,
                                            mybir.EngineType.DVE)):
                    _build_body(cx, phases)
    nc.finalize()
    return nc


_NC_CACHE = {}


def _get_nc(repeat=1, phases=4):
    key = (repeat, phases)
    if key not in _NC_CACHE:
        _NC_CACHE[key] = build_nc(repeat, phases)
    return _NC_CACHE[key]


def kernel(**inputs: np.ndarray) -> np.ndarray:
    nc = _get_nc()
    x_in = np.ascontiguousarray(inputs["x_in"], dtype=np.float32)
    shared = {}
    for name in W_NAMES + B_NAMES:
        shared[name] = np.ascontiguousarray(inputs[name], dtype=np.float32)
    in_maps = [dict(shared, x_in=x_in[b]) for b in range(NCORES)]
    res = run_bass_kernel_spmd(nc, in_maps, core_ids=list(range(NCORES)))
    return np.stack([res.results[b]["out"] for b in range(NCORES)], axis=0)
